# revision 4
# baseline (speedup 1.0000x reference)
"""Block-sparse self-attention (block=20, heads=4) on 8 TRN2 NeuronCores.

Strategy: data-parallel over batch B=32 -> 4 sequences per core; weights
replicated. Fully fused on-chip pipeline per 480-token chunk (no HBM
round-trips for qkv/attention intermediates):

  x^T (host-pretransposed, bf16)  --DMA-->  x_fm [d,t] SBUF
  q,k: feature-major projections (PE, lhsT=W^T chunks, rhs=x_fm)
  v:   token-major projection    (PE, lhsT=x_fm, rhs=Wv^T)
  per 120-token subtile (6 blocks):
    S = (q*s)^T k per head, 2-head row-packed matmuls -> PSUM
    exp on ACT (no max-subtraction: |S| <~ 20 so exp is safe in fp32/bf16)
    block-diag 0/1 mask multiply, row-sum, reciprocal, scale  (DVE)
    A^T via PE transpose; AV col-packed -> o feature-major
    out-proj: lhsT=o_fm, rhs=Wo^T -> y token-major PSUM -> SBUF -> DMA out

All matmuls bf16 inputs with fp32 PSUM accumulation. Biases are applied
generally (ACT per-partition bias for q/k; rank-1 ones-matmul for v and y).
"""

import numpy as np
import ml_dtypes

import concourse.bass as bass
import concourse.mybir as mybir
import concourse.tile as tile
from concourse import bacc
from concourse.bass_utils import run_bass_kernel_spmd

F32 = mybir.dt.float32
BF16 = mybir.dt.bfloat16

B, T, D = 32, 4000, 256
BS = 20            # attention block size
H = 4              # heads
HD = D // H        # 64
NCORES = 8
BPC = B // NCORES  # sequences per core

CHUNK = 480        # tokens per chunk (24 blocks)
SUB = 120          # tokens per subtile (6 blocks), M-dim of attention matmuls


def _chunks_for(t_total):
    """Yield (t0, [subtile sizes]) covering t_total tokens."""
    out = []
    t0 = 0
    while t0 < t_total:
        ch = min(CHUNK, t_total - t0)
        subs = []
        off = 0
        while off < ch:
            subs.append(min(SUB, ch - off))
            off += subs[-1]
        out.append((t0, subs))
        t0 += ch
    return out


def build_program(bpc=BPC, t_total=T):
    nc = bacc.Bacc("TRN2", target_bir_lowering=False, debug=False,
                   num_devices=NCORES)

    # ---- DRAM I/O ----
    xT = nc.dram_tensor("xT", [bpc, D, t_total], BF16, kind="ExternalInput")
    wqkT = nc.dram_tensor("wqkT", [D, 2 * D], BF16, kind="ExternalInput")
    wvT = nc.dram_tensor("wvT", [D, D], BF16, kind="ExternalInput")
    woT = nc.dram_tensor("woT", [D, D], BF16, kind="ExternalInput")
    bqk = nc.dram_tensor("bqk", [4, 128], F32, kind="ExternalInput")
    bv = nc.dram_tensor("bv", [1, D], BF16, kind="ExternalInput")
    by = nc.dram_tensor("by", [1, D], BF16, kind="ExternalInput")
    maskc = nc.dram_tensor("maskc", [SUB, 2, SUB], BF16, kind="ExternalInput")
    onesc = nc.dram_tensor("onesc", [1, SUB], BF16, kind="ExternalInput")
    identc = nc.dram_tensor("identc", [128, 128], BF16, kind="ExternalInput")
    y = nc.dram_tensor("y", [bpc, t_total, D], F32, kind="ExternalOutput")

    xT_r = xT.rearrange("b (dk p) t -> b p dk t", p=128)
    wqkT_r = wqkT.rearrange("(dk p) e -> p dk e", p=128)
    wvT_r = wvT.rearrange("(dk p) c -> p dk c", p=128)
    woT_r = woT.rearrange("(ek p) c -> p ek c", p=128)
    bqk_r = bqk.rearrange("c p -> p c")

    with tile.TileContext(nc) as tc:
        with (
            tc.tile_pool(name="consts", bufs=1) as cpool,
            tc.tile_pool(name="xf", bufs=2) as xpool,
            tc.tile_pool(name="qk", bufs=2) as qkpool,
            tc.tile_pool(name="att", bufs=3) as apool,
            tc.tile_pool(name="out", bufs=3) as opool,
            tc.tile_pool(name="ps", bufs=8, space="PSUM") as pspool,
        ):
            # ---- constants in SBUF ----
            wqk_sb = cpool.tile([128, 2, 2 * D], BF16, tag="wqk")
            nc.sync.dma_start(out=wqk_sb, in_=wqkT_r)
            wv_sb = cpool.tile([128, 2, D], BF16, tag="wv")
            nc.sync.dma_start(out=wv_sb, in_=wvT_r)
            wo_sb = cpool.tile([128, 2, D], BF16, tag="wo")
            nc.sync.dma_start(out=wo_sb, in_=woT_r)
            bqk_sb = cpool.tile([128, 4], F32, tag="bqk")
            nc.sync.dma_start(out=bqk_sb, in_=bqk_r)
            bv_sb = cpool.tile([1, D], BF16, tag="bv")
            nc.sync.dma_start(out=bv_sb, in_=bv[:, :])
            by_sb = cpool.tile([1, D], BF16, tag="by")
            nc.sync.dma_start(out=by_sb, in_=by[:, :])
            mask_sb = cpool.tile([SUB, 2, SUB], BF16, tag="mask")
            nc.sync.dma_start(out=mask_sb, in_=maskc[:, :, :])
            ones_sb = cpool.tile([1, SUB], BF16, tag="ones")
            nc.sync.dma_start(out=ones_sb, in_=onesc[:, :])
            id_sb = cpool.tile([128, 128], BF16, tag="ident")
            nc.sync.dma_start(out=id_sb, in_=identc[:, :])

            for b in range(bpc):
                for (t0, subs) in _chunks_for(t_total):
                    ch = sum(subs)
                    # ---- load x^T chunk: [128, 2, ch] bf16 ----
                    xfm = xpool.tile([128, 2, CHUNK], BF16, tag="xfm")
                    nc.sync.dma_start(out=xfm[:, :, :ch],
                                      in_=xT_r[b, :, :, t0:t0 + ch])

                    # ---- q, k feature-major projections ----
                    # pc 0,1 = q e-chunks; 2,3 = k e-chunks
                    qk_sb = []
                    for pc in range(4):
                        ps = pspool.tile([128, CHUNK], F32, tag="ps")
                        for dk in range(2):
                            nc.tensor.matmul(
                                ps[:, :ch],
                                wqk_sb[:, dk, pc * 128:(pc + 1) * 128],
                                xfm[:, dk, :ch],
                                start=(dk == 0), stop=(dk == 1),
                            )
                        sb = qkpool.tile([128, CHUNK], BF16, tag=f"qk{pc}")
                        scale = 0.125 if pc < 2 else 1.0
                        nc.scalar.activation(
                            sb[:, :ch], ps[:, :ch],
                            mybir.ActivationFunctionType.Identity,
                            bias=bqk_sb[:, pc:pc + 1], scale=scale,
                        )
                        qk_sb.append(sb)

                    # ---- v token-major projection (per subtile) ----
                    vtm = xpool.tile([SUB, len(subs), D], BF16, tag="vtm")
                    off = 0
                    for si, s in enumerate(subs):
                        vps = pspool.tile([SUB, D], F32, tag="ps")
                        for dk in range(2):
                            nc.tensor.matmul(
                                vps[:s, :],
                                xfm[:, dk, off:off + s],
                                wv_sb[:, dk, :],
                                start=(dk == 0), stop=False,
                            )
                        nc.tensor.matmul(
                            vps[:s, :], ones_sb[:, :s], bv_sb[:, :],
                            start=False, stop=True,
                        )
                        nc.vector.tensor_copy(vtm[:s, si, :], vps[:s, :])
                        off += s

                    # ---- attention + out-proj per subtile ----
                    off = 0
                    for si, s in enumerate(subs):
                        tw = slice(off, off + s)
                        # scores: row-packed pairs; bank A = heads 0,2 ; B = 1,3
                        sA = pspool.tile([SUB, 2, SUB], F32, tag="ps")
                        sB = pspool.tile([SUB, 2, SUB], F32, tag="ps")
                        for h in range(H):
                            ec = h // 2          # q/k e-chunk
                            rp = (h % 2) * 64    # partition base within chunk
                            dst = (sA if h % 2 == 0 else sB)
                            nc.tensor.matmul(
                                dst[:s, ec, :s],
                                qk_sb[ec][rp:rp + 64, tw],
                                qk_sb[2 + ec][rp:rp + 64, tw],
                                start=True, stop=True,
                                tile_position=(rp, 0),
                            )
                        # exp -> bf16 SBUF
                        eA = apool.tile([SUB, 2, SUB], BF16, tag="eA")
                        eB = apool.tile([SUB, 2, SUB], BF16, tag="eB")
                        nc.scalar.activation(eA[:s, :, :s], sA[:s, :, :s],
                                             mybir.ActivationFunctionType.Exp)
                        nc.scalar.activation(eB[:s, :, :s], sB[:s, :, :s],
                                             mybir.ActivationFunctionType.Exp)
                        # mask, row-sums, reciprocal
                        den = apool.tile([SUB, 4], F32, tag="den")
                        rec = apool.tile([SUB, 4], F32, tag="rec")
                        for ti, e in ((0, eA), (1, eB)):
                            nc.vector.tensor_mul(e[:s, :, :s], e[:s, :, :s],
                                                 mask_sb[:s, :, :s])
                            nc.vector.reduce_sum(den[:s, 2 * ti:2 * ti + 2],
                                                 e[:s, :, :s],
                                                 axis=mybir.AxisListType.X)
                        nc.vector.reciprocal(rec[:s, :], den[:s, :])
                        # A = E * recip  (per head slot)
                        for ti, e in ((0, eA), (1, eB)):
                            for sl in range(2):
                                nc.vector.tensor_scalar_mul(
                                    e[:s, sl, :s], e[:s, sl, :s],
                                    rec[:s, 2 * ti + sl:2 * ti + sl + 1])
                        # A^T via PE transpose; head h -> (tile h%2, slot h//2)
                        atps = pspool.tile([SUB, 4, SUB], BF16, tag="ps")
                        for h in range(H):
                            e = eA if h % 2 == 0 else eB
                            nc.tensor.transpose(atps[:s, h, :s],
                                                e[:s, h // 2, :s],
                                                id_sb[:s, :s])
                        at_sb = apool.tile([SUB, 4, SUB], BF16, tag="at")
                        nc.scalar.activation(at_sb[:s, :, :s], atps[:s, :, :s],
                                             mybir.ActivationFunctionType.Copy)
                        # AV: col-packed pairs -> o feature-major [128, s] x2
                        o_sb = opool.tile([128, 2, SUB], BF16, tag="osb")
                        for pair in range(2):
                            ops = pspool.tile([128, SUB], F32, tag="ps")
                            for hh in range(2):
                                h = pair * 2 + hh
                                cp = hh * 64
                                nc.tensor.matmul(
                                    ops[cp:cp + 64, :s],
                                    vtm[:s, si, h * HD:(h + 1) * HD],
                                    at_sb[:s, h, :s],
                                    start=True, stop=True,
                                    tile_position=(0, cp),
                                )
                            nc.vector.tensor_copy(o_sb[:, pair, :s],
                                                  ops[:, :s])
                        # out-proj -> y token-major
                        yps = pspool.tile([SUB, D], F32, tag="ps")
                        for ec in range(2):
                            nc.tensor.matmul(
                                yps[:s, :],
                                o_sb[:, ec, :s],
                                wo_sb[:, ec, :],
                                start=(ec == 0), stop=False,
                            )
                        nc.tensor.matmul(
                            yps[:s, :], ones_sb[:, :s], by_sb[:, :],
                            start=False, stop=True,
                        )
                        y_sb = opool.tile([SUB, D], F32, tag="ysb")
                        nc.vector.tensor_copy(y_sb[:s, :], yps[:s, :])
                        nc.sync.dma_start(out=y[b, t0 + off:t0 + off + s, :],
                                          in_=y_sb[:s, :])
                        off += s

    nc.compile()
    return nc


_PROG = {}


def _get_program(bpc, t_total):
    key = (bpc, t_total)
    if key not in _PROG:
        _PROG[key] = build_program(bpc, t_total)
    return _PROG[key]


def _bf(a):
    return np.ascontiguousarray(a.astype(ml_dtypes.bfloat16))


def kernel(x, in_proj_w, in_proj_b, out_proj_w, out_proj_b):
    x = np.asarray(x, dtype=np.float32)
    in_proj_w = np.asarray(in_proj_w, dtype=np.float32)
    in_proj_b = np.asarray(in_proj_b, dtype=np.float32)
    out_proj_w = np.asarray(out_proj_w, dtype=np.float32)
    out_proj_b = np.asarray(out_proj_b, dtype=np.float32)

    b_total, t_total, d = x.shape
    bpc = b_total // NCORES
    nc = _get_program(bpc, t_total)

    # host-side prep (shared weights)
    wqkT = _bf(in_proj_w[:2 * D].T)                      # [D, 512]
    wvT = _bf(in_proj_w[2 * D:].T)                       # [D, 256]
    woT = _bf(out_proj_w.T)                              # [D, 256]
    bqk = np.ascontiguousarray(
        in_proj_b[:2 * D].reshape(4, 128).astype(np.float32))
    bqk[:2] *= 0.125                                     # q bias pre-scaled
    bv = _bf(in_proj_b[2 * D:].reshape(1, D))
    by = _bf(out_proj_b.reshape(1, D))
    blk = np.arange(SUB) // BS
    mask1 = (blk[:, None] == blk[None, :]).astype(np.float32)
    maskc = _bf(np.repeat(mask1[:, None, :], 2, axis=1))  # [SUB, 2, SUB]
    onesc = _bf(np.ones((1, SUB), np.float32))
    identc = _bf(np.eye(128, dtype=np.float32))

    in_maps = []
    for c in range(NCORES):
        xs = x[c * bpc:(c + 1) * bpc]                    # [bpc, T, D]
        xT = _bf(xs.transpose(0, 2, 1))                  # [bpc, D, T]
        in_maps.append({
            "xT": xT, "wqkT": wqkT, "wvT": wvT, "woT": woT,
            "bqk": bqk, "bv": bv, "by": by,
            "maskc": maskc, "onesc": onesc, "identc": identc,
        })

    global _last_in_maps
    _last_in_maps = in_maps
    res = run_bass_kernel_spmd(nc, in_maps, core_ids=list(range(NCORES)))
    out = np.concatenate([res.results[c]["y"] for c in range(NCORES)], axis=0)
    return out.astype(np.float32)


_last_in_maps = None


# revision 8
# speedup vs baseline: 1.2673x; 1.2673x over previous
"""Block-sparse self-attention (block=20, heads=4) on 8 TRN2 NeuronCores.

Strategy: data-parallel over batch B=32 -> 4 sequences per core; weights
replicated. Fully fused on-chip pipeline per 480-token chunk (no HBM
round-trips for qkv/attention intermediates):

  x^T (host-pretransposed, bf16)  --DMA-->  x_fm [d,t] SBUF
  q,k: feature-major projections (PE, lhsT=W^T chunks, rhs=x_fm)
  v:   token-major projection    (PE, lhsT=x_fm, rhs=Wv^T)
  per 120-token subtile (6 blocks):
    S = (q*s)^T k per head, 2-head row-packed matmuls -> PSUM
    exp on ACT (no max-subtraction: |S| <~ 20 so exp is safe in fp32/bf16)
    block-diag 0/1 mask multiply, row-sum, reciprocal, scale  (DVE)
    A^T via PE transpose; AV col-packed -> o feature-major
    out-proj: lhsT=o_fm, rhs=Wo^T -> y token-major PSUM -> SBUF -> DMA out

All matmuls bf16 inputs with fp32 PSUM accumulation. Biases are applied
generally (ACT per-partition bias for q/k; rank-1 ones-matmul for v and y).
"""

import numpy as np
import ml_dtypes

import concourse.bass as bass
import concourse.mybir as mybir
import concourse.tile as tile
from concourse import bacc
from concourse.bass_utils import run_bass_kernel_spmd

F32 = mybir.dt.float32
BF16 = mybir.dt.bfloat16

B, T, D = 32, 4000, 256
BS = 20            # attention block size
H = 4              # heads
HD = D // H        # 64
NCORES = 8
BPC = B // NCORES  # sequences per core

CHUNK = 480        # tokens per chunk (24 blocks)
SUB = 120          # tokens per subtile (6 blocks), M-dim of attention matmuls


def _chunks_for(t_total):
    """Yield (t0, [subtile sizes]) covering t_total tokens."""
    out = []
    t0 = 0
    while t0 < t_total:
        ch = min(CHUNK, t_total - t0)
        subs = []
        off = 0
        while off < ch:
            subs.append(min(SUB, ch - off))
            off += subs[-1]
        out.append((t0, subs))
        t0 += ch
    return out


def build_program(bpc=BPC, t_total=T):
    nc = bacc.Bacc("TRN2", target_bir_lowering=False, debug=False,
                   num_devices=NCORES)

    # ---- DRAM I/O ----
    xT = nc.dram_tensor("xT", [bpc, D, t_total], BF16, kind="ExternalInput")
    wqkT = nc.dram_tensor("wqkT", [D, 2 * D], BF16, kind="ExternalInput")
    wvT = nc.dram_tensor("wvT", [D, D], BF16, kind="ExternalInput")
    woT = nc.dram_tensor("woT", [D, D], BF16, kind="ExternalInput")
    bqk = nc.dram_tensor("bqk", [4, 128], F32, kind="ExternalInput")
    bv = nc.dram_tensor("bv", [1, D], BF16, kind="ExternalInput")
    by = nc.dram_tensor("by", [1, D], BF16, kind="ExternalInput")
    maskc = nc.dram_tensor("maskc", [SUB, 4, SUB], BF16, kind="ExternalInput")
    onesc = nc.dram_tensor("onesc", [1, SUB], BF16, kind="ExternalInput")
    identc = nc.dram_tensor("identc", [128, 128], BF16, kind="ExternalInput")
    y = nc.dram_tensor("y", [bpc, t_total, D], F32, kind="ExternalOutput")

    xT_r = xT.rearrange("b (dk p) t -> b p dk t", p=128)
    wqkT_r = wqkT.rearrange("(dk p) e -> p dk e", p=128)
    wvT_r = wvT.rearrange("(dk p) c -> p dk c", p=128)
    woT_r = woT.rearrange("(ek p) c -> p ek c", p=128)
    bqk_r = bqk.rearrange("c p -> p c")

    with tile.TileContext(nc) as tc:
        with (
            tc.tile_pool(name="consts", bufs=1) as cpool,
            tc.tile_pool(name="xf", bufs=2) as xpool,
            tc.tile_pool(name="qk", bufs=2) as qkpool,
            tc.tile_pool(name="att", bufs=3) as apool,
            tc.tile_pool(name="out", bufs=3) as opool,
            tc.tile_pool(name="ps", bufs=8, space="PSUM") as pspool,
        ):
            # ---- constants in SBUF ----
            wqk_sb = cpool.tile([128, 2, 2 * D], BF16, tag="wqk")
            nc.sync.dma_start(out=wqk_sb, in_=wqkT_r)
            wv_sb = cpool.tile([128, 2, D], BF16, tag="wv")
            nc.sync.dma_start(out=wv_sb, in_=wvT_r)
            wo_sb = cpool.tile([128, 2, D], BF16, tag="wo")
            nc.sync.dma_start(out=wo_sb, in_=woT_r)
            bqk_sb = cpool.tile([128, 4], F32, tag="bqk")
            nc.sync.dma_start(out=bqk_sb, in_=bqk_r)
            bv_sb = cpool.tile([1, D], BF16, tag="bv")
            nc.sync.dma_start(out=bv_sb, in_=bv[:, :])
            by_sb = cpool.tile([1, D], BF16, tag="by")
            nc.sync.dma_start(out=by_sb, in_=by[:, :])
            mask_sb = cpool.tile([SUB, 4, SUB], BF16, tag="mask")
            nc.sync.dma_start(out=mask_sb, in_=maskc[:, :, :])
            ones_sb = cpool.tile([1, SUB], BF16, tag="ones")
            nc.sync.dma_start(out=ones_sb, in_=onesc[:, :])
            id_sb = cpool.tile([128, 128], BF16, tag="ident")
            nc.sync.dma_start(out=id_sb, in_=identc[:, :])

            for b in range(bpc):
                for (t0, subs) in _chunks_for(t_total):
                    ch = sum(subs)
                    # ---- load x^T chunk: [128, 2, ch] bf16 ----
                    xfm = xpool.tile([128, 2, CHUNK], BF16, tag="xfm")
                    nc.sync.dma_start(out=xfm[:, :, :ch],
                                      in_=xT_r[b, :, :, t0:t0 + ch])

                    # ---- q, k feature-major projections ----
                    # pc 0,1 = q e-chunks; 2,3 = k e-chunks
                    qk_sb = []
                    for pc in range(4):
                        ps = pspool.tile([128, CHUNK], F32, tag="ps")
                        for dk in range(2):
                            nc.tensor.matmul(
                                ps[:, :ch],
                                wqk_sb[:, dk, pc * 128:(pc + 1) * 128],
                                xfm[:, dk, :ch],
                                start=(dk == 0), stop=(dk == 1),
                            )
                        sb = qkpool.tile([128, CHUNK], BF16, tag=f"qk{pc}")
                        scale = 0.125 if pc < 2 else 1.0
                        nc.scalar.activation(
                            sb[:, :ch], ps[:, :ch],
                            mybir.ActivationFunctionType.Identity,
                            bias=bqk_sb[:, pc:pc + 1], scale=scale,
                        )
                        qk_sb.append(sb)

                    # ---- v token-major projection (per subtile) ----
                    vtm = xpool.tile([SUB, len(subs), D], BF16, tag="vtm")
                    off = 0
                    for si, s in enumerate(subs):
                        vps = pspool.tile([SUB, D], F32, tag="ps")
                        for dk in range(2):
                            nc.tensor.matmul(
                                vps[:s, :],
                                xfm[:, dk, off:off + s],
                                wv_sb[:, dk, :],
                                start=(dk == 0), stop=False,
                            )
                        nc.tensor.matmul(
                            vps[:s, :], ones_sb[:, :s], bv_sb[:, :],
                            start=False, stop=True,
                        )
                        nc.vector.tensor_copy(vtm[:s, si, :], vps[:s, :])
                        off += s

                    # ---- attention + out-proj per subtile ----
                    off = 0
                    for si, s in enumerate(subs):
                        tw = slice(off, off + s)
                        # scores: 2-head row-packed; bank A = heads 0,2 ; B = 1,3
                        sA = pspool.tile([SUB, 2, SUB], F32, tag="ps")
                        sB = pspool.tile([SUB, 2, SUB], F32, tag="ps")
                        for h in range(H):
                            ec = h // 2          # q/k e-chunk
                            rp = (h % 2) * 64    # partition base within chunk
                            dst = (sA if h % 2 == 0 else sB)
                            nc.tensor.matmul(
                                dst[:s, ec, :s],
                                qk_sb[ec][rp:rp + 64, tw],
                                qk_sb[2 + ec][rp:rp + 64, tw],
                                start=True, stop=True,
                                tile_position=(rp, 0),
                            )
                        # exp -> bf16 SBUF; ee layout [s, (hA0,hA1,hB0,hB1), s]
                        # slot mapping: head h -> slot 2*(h%2) + h//2
                        ee = apool.tile([SUB, 4, SUB], BF16, tag="ee")
                        nc.scalar.activation(ee[:s, 0:2, :s], sA[:s, :, :s],
                                             mybir.ActivationFunctionType.Exp)
                        nc.scalar.activation(ee[:s, 2:4, :s], sB[:s, :, :s],
                                             mybir.ActivationFunctionType.Exp)
                        # mask, row-sums, reciprocal, scale
                        den = apool.tile([SUB, 4], F32, tag="den")
                        rec = apool.tile([SUB, 4], F32, tag="rec")
                        nc.vector.tensor_mul(ee[:s, :, :s], ee[:s, :, :s],
                                             mask_sb[:s, :, :s])
                        nc.vector.reduce_sum(den[:s, :], ee[:s, :, :s],
                                             axis=mybir.AxisListType.X)
                        nc.vector.reciprocal(rec[:s, :], den[:s, :])
                        for sl in range(4):
                            nc.vector.tensor_scalar_mul(
                                ee[:s, sl, :s], ee[:s, sl, :s],
                                rec[:s, sl:sl + 1])
                        # A^T via PE transpose (head h at slot 2*(h%2)+h//2)
                        atps = pspool.tile([SUB, 4, SUB], BF16, tag="ps")
                        for h in range(H):
                            sl = 2 * (h % 2) + h // 2
                            nc.tensor.transpose(atps[:s, h, :s],
                                                ee[:s, sl, :s],
                                                id_sb[:s, :s])
                        at_sb = apool.tile([SUB, 4, SUB], BF16, tag="at")
                        nc.scalar.activation(at_sb[:s, :, :s], atps[:s, :, :s],
                                             mybir.ActivationFunctionType.Copy)
                        # AV: col-packed pairs -> o feature-major, 1 bank/pair
                        o_sb = opool.tile([128, 2, SUB], BF16, tag="osb")
                        for pair in range(2):
                            ops = pspool.tile([128, SUB], F32, tag="ps")
                            for hh in range(2):
                                h = pair * 2 + hh
                                cp = hh * 64
                                nc.tensor.matmul(
                                    ops[cp:cp + 64, :s],
                                    vtm[:s, si, h * HD:(h + 1) * HD],
                                    at_sb[:s, h, :s],
                                    start=True, stop=True,
                                    tile_position=(0, cp),
                                )
                            nc.vector.tensor_copy(o_sb[:, pair, :s],
                                                  ops[:, :s])
                        # out-proj -> y token-major
                        yps = pspool.tile([SUB, D], F32, tag="ps")
                        for ec in range(2):
                            nc.tensor.matmul(
                                yps[:s, :],
                                o_sb[:, ec, :s],
                                wo_sb[:, ec, :],
                                start=(ec == 0), stop=False,
                            )
                        nc.tensor.matmul(
                            yps[:s, :], ones_sb[:, :s], by_sb[:, :],
                            start=False, stop=True,
                        )
                        y_sb = opool.tile([SUB, D], F32, tag="ysb")
                        nc.vector.tensor_copy(y_sb[:s, :], yps[:s, :])
                        nc.sync.dma_start(out=y[b, t0 + off:t0 + off + s, :],
                                          in_=y_sb[:s, :])
                        off += s

    nc.compile()
    return nc


_PROG = {}


def _get_program(bpc, t_total):
    key = (bpc, t_total)
    if key not in _PROG:
        _PROG[key] = build_program(bpc, t_total)
    return _PROG[key]


def _bf(a):
    return np.ascontiguousarray(a.astype(ml_dtypes.bfloat16))


def kernel(x, in_proj_w, in_proj_b, out_proj_w, out_proj_b):
    x = np.asarray(x, dtype=np.float32)
    in_proj_w = np.asarray(in_proj_w, dtype=np.float32)
    in_proj_b = np.asarray(in_proj_b, dtype=np.float32)
    out_proj_w = np.asarray(out_proj_w, dtype=np.float32)
    out_proj_b = np.asarray(out_proj_b, dtype=np.float32)

    b_total, t_total, d = x.shape
    bpc = b_total // NCORES
    nc = _get_program(bpc, t_total)

    # host-side prep (shared weights)
    wqkT = _bf(in_proj_w[:2 * D].T)                      # [D, 512]
    wvT = _bf(in_proj_w[2 * D:].T)                       # [D, 256]
    woT = _bf(out_proj_w.T)                              # [D, 256]
    bqk = np.ascontiguousarray(
        in_proj_b[:2 * D].reshape(4, 128).astype(np.float32))
    bqk[:2] *= 0.125                                     # q bias pre-scaled
    bv = _bf(in_proj_b[2 * D:].reshape(1, D))
    by = _bf(out_proj_b.reshape(1, D))
    blk = np.arange(SUB) // BS
    mask1 = (blk[:, None] == blk[None, :]).astype(np.float32)
    maskc = _bf(np.repeat(mask1[:, None, :], 4, axis=1))  # [SUB, 4, SUB]
    onesc = _bf(np.ones((1, SUB), np.float32))
    identc = _bf(np.eye(128, dtype=np.float32))

    in_maps = []
    for c in range(NCORES):
        xs = x[c * bpc:(c + 1) * bpc]                    # [bpc, T, D]
        xT = _bf(xs.transpose(0, 2, 1))                  # [bpc, D, T]
        in_maps.append({
            "xT": xT, "wqkT": wqkT, "wvT": wvT, "woT": woT,
            "bqk": bqk, "bv": bv, "by": by,
            "maskc": maskc, "onesc": onesc, "identc": identc,
        })

    global _last_in_maps
    _last_in_maps = in_maps
    res = run_bass_kernel_spmd(nc, in_maps, core_ids=list(range(NCORES)))
    out = np.concatenate([res.results[c]["y"] for c in range(NCORES)], axis=0)
    return out.astype(np.float32)


_last_in_maps = None
